# revision 1
# baseline (speedup 1.0000x reference)
"""nn_BinaryMoSLinear Trainium2 kernel: 8 NeuronCores, data-parallel over tokens.

kernel(**inputs) takes the FULL reference.setup_inputs() tensors and returns
the FULL [4, 2048, 4096] f32 output. Tokens are sharded 1024/core; weight,
bias, gate and channel scales are replicated. Each core runs the whole
router/softmax/scale/binarized-matmul pipeline independently (no
collectives); the weight streams through SBUF exactly once (single-pass,
8 PSUM accumulation banks), all matmuls in fp32r.
"""
from contextlib import ExitStack

import concourse.bass as bass
import concourse.mybir as mybir

F32 = mybir.dt.float32
F32R = mybir.dt.float32r
AF = mybir.ActivationFunctionType
OP = mybir.AluOpType


def f32(ap):
    return ap.bitcast(F32)


def build_moe8(ctx, tc, outs, ins, cfg):
    nc = tc.nc
    H, O, Nc, E = cfg["H"], cfg["O"], cfg["Nc"], cfg["E"]
    half, ow = cfg["half"], cfg["ow"]
    NH = H // 128
    HALVES = Nc // half
    J = half // 128
    OC = O // ow
    y = outs["y"]

    pool = ctx.enter_context(tc.tile_pool(name="sb", bufs=1))

    gw_all = pool.tile([128, E * NH], F32R, name="gw_all", tag="gw", bufs=1)
    for h in range(NH):
        nc.sync.dma_start(gw_all[:, h * E:(h + 1) * E],
                          ins["gwT"][h * 128:(h + 1) * 128, :])
    ones_e1 = pool.tile([E, 1], F32R, name="ones_e1", tag="ones_e1", bufs=1)
    nc.sync.dma_start(ones_e1[:], ins["ones_e"][:, 0:1])
    ones_1e = pool.tile([1, E], F32R, name="ones_1e", tag="ones_1e", bufs=1)
    nc.sync.dma_start(ones_1e[:], ins["ones_e"][0:1, :])
    eye_e = pool.tile([E, E], F32R, name="eye_e", tag="eye_e", bufs=1)
    nc.sync.dma_start(eye_e[:], ins["eye_e"][:, :])

    xs = {}
    r_nj = {}
    wst_map = {}

    def slab_pair(oc, p):
        o0 = oc * ow
        wst = pool.tile([128, 2 * ow], F32R, name=f"wsp{oc}_{p}",
                        tag="wst", bufs=4)
        for k in range(2):
            h = 2 * p + k
            nc.sync.dma_start(wst[:, k * ow:(k + 1) * ow],
                              ins["wT"][h * 128:(h + 1) * 128, o0:o0 + ow])
        nc.scalar.activation(wst[:], f32(wst[:]), AF.Sign)
        wst_map[(oc, 2 * p)] = wst[:, 0:ow]
        wst_map[(oc, 2 * p + 1)] = wst[:, ow:2 * ow]

    def prologue_slab(p):
        if p < 3:
            slab_pair(0, p)

    # ---- prologue: router + in-scale for both halves (2-buf PSUM pool).
    # S(half0) is interleaved with R(half1) (independent, different
    # engines), and the first signed weight slab pairs prefetch during
    # S(half1), to compress the PE-sparse prologue. ----
    rts = {}

    def r_step(psm, hf, h):
        base = hf * half
        xt = pool.tile([128, half], F32R, name=f"xr{hf}_{h}", tag="xt", bufs=5)
        nc.sync.dma_start(xt[:], ins["xT"][h * 128:(h + 1) * 128,
                                           base:base + half])
        if hf not in logits_t:
            logits_t[hf] = psm.tile([E, half], F32, name=f"logits{hf}", tag="psm")
        nc.tensor.matmul(logits_t[hf][:], gw_all[:, h * E:(h + 1) * E], xt[:],
                         start=(h == 0), stop=(h == NH - 1))

    def r_finish(psm, hf):
        logits = logits_t[hf]
        ex = pool.tile([E, half], F32R, name=f"ex{hf}", tag="exr", bufs=2)
        nc.scalar.activation(ex[:], logits[:], AF.Exp)
        ssum = psm.tile([1, half], F32, name=f"ssum{hf}", tag="psm")
        nc.tensor.matmul(ssum[:], ones_e1[:], ex[:], start=True, stop=True)
        rcp = pool.tile([1, half], F32R, name=f"rcp{hf}", tag="exr", bufs=2)
        with nc.allow_low_precision(reason="fp32r feeds PE broadcast matmul"):
            nc.vector.reciprocal(rcp[:], ssum[:])
        bc = psm.tile([E, half], F32, name=f"bc{hf}", tag="psm")
        nc.tensor.matmul(bc[:], ones_1e[:], rcp[:], start=True, stop=True)
        rt = pool.tile([E, half], F32R, name=f"rt{hf}", tag="rt", bufs=2)
        nc.vector.tensor_tensor(rt[:], f32(ex[:]), bc[:], OP.mult)
        rts[hf] = rt
        for j in range(J):
            rtp = psm.tile([128, E], F32R, name=f"rtp{hf}_{j}", tag="psm")
            nc.tensor.transpose(rtp[:], rt[:, j * 128:(j + 1) * 128], eye_e[:])
            rj = pool.tile([128, E], F32, name=f"rnj{hf}_{j}",
                           tag=f"rnj{hf}_{j}", bufs=1)
            nc.vector.tensor_copy(rj[:], f32(rtp[:]))
            r_nj[(hf, j)] = rj

    def s_step(psm, hf, h):
        base = hf * half
        xt = pool.tile([128, half], F32R, name=f"xs_in{hf}_{h}", tag="xt", bufs=5)
        nc.sync.dma_start(xt[:], ins["xT"][h * 128:(h + 1) * 128,
                                           base:base + half])
        icst = pool.tile([E, 128], F32R, name=f"ics{hf}_{h}", tag="ics", bufs=8)
        nc.sync.dma_start(icst[:], ins["ics"][:, h * 128:(h + 1) * 128])
        isp = psm.tile([128, half], F32, name=f"isp{hf}_{h}", tag="psm")
        nc.tensor.matmul(isp[:], icst[:], rts[hf][:], start=True, stop=True)
        x_s = pool.tile([128, half], F32R, name=f"xs{hf}_{h}",
                        tag=f"xs{hf}_{h}", bufs=1)
        nc.vector.tensor_tensor(x_s[:], f32(xt[:]), isp[:], OP.mult)
        xs[(hf, h)] = x_s

    logits_t = {}
    with tc.tile_pool(name="psm", bufs=3, space="PSUM") as psm:
        for h in range(NH):
            r_step(psm, 0, h)
        r_finish(psm, 0)
        for h in range(NH):
            s_step(psm, 0, h)
            r_step(psm, 1, h)
        r_finish(psm, 1)
        for h in range(NH):
            s_step(psm, 1, h)
            if HALVES == 2 and h % 8 == 7:
                prologue_slab(h // 8)

    # ---- main phase: one pass over the weight, 8 PSUM banks ----
    with tc.tile_pool(name="psmain", bufs=8, space="PSUM") as psmain:
        for oc in range(OC):
            o0 = oc * ow
            bst = pool.tile([128, ow], F32, name=f"bias{oc}", tag="bias", bufs=2)
            nc.sync.dma_start(bst[:], ins["bias2"][:, o0:o0 + ow])
            ocsb = []
            for e in range(E):
                ob = pool.tile([128, ow], F32, name=f"ocsb{oc}_{e}", tag="ocsb", bufs=4)
                nc.sync.dma_start(ob[:], ins["ocsb"][e * 128:(e + 1) * 128,
                                                     o0:o0 + ow])
                ocsb.append(ob)
            mains = {}
            for hf in range(HALVES):
                for j in range(J):
                    mains[(hf, j)] = psmain.tile(
                        [128, ow], F32, name=f"mp{oc}_{hf}_{j}", tag="mps")
            # out-scale chunks precomputed on DVE, spread through the h-loop
            os_sb = {}

            def os_chunk(hf, j):
                rj = r_nj[(hf, j)]
                t = pool.tile([128, ow], F32, name=f"t{oc}_{hf}_{j}",
                              tag="ossb", bufs=8)
                nc.vector.tensor_scalar_mul(t[:], ocsb[0][:], rj[:, 0:1])
                for e in range(1, E):
                    nc.vector.scalar_tensor_tensor(
                        t[:], ocsb[e][:], rj[:, e:e + 1], t[:], OP.mult, OP.add)
                os_sb[(hf, j)] = t

            chunks = [(hf, j) for hf in range(HALVES) for j in range(J)]
            ci = 0
            for h in range(NH):
                if (oc, h) not in wst_map:
                    slab_pair(oc, h // 2)
                wst = wst_map.pop((oc, h))
                for hf in range(HALVES):
                    for j in range(J):
                        nc.tensor.matmul(mains[(hf, j)][:],
                                         xs[(hf, h)][:, j * 128:(j + 1) * 128],
                                         wst[:], start=(h == 0), stop=(h == NH - 1))
                if h % (NH // len(chunks)) == (NH // len(chunks)) - 1 and ci < len(chunks):
                    os_chunk(*chunks[ci])
                    ci += 1
            while ci < len(chunks):
                os_chunk(*chunks[ci]); ci += 1
            for hf in range(HALVES):
                base = hf * half
                for j in range(J):
                    yt = pool.tile([128, ow], F32, name=f"yt{oc}_{hf}_{j}",
                                   tag="yt", bufs=3)
                    nc.vector.tensor_tensor(yt[:], mains[(hf, j)][:],
                                            os_sb[(hf, j)][:], OP.mult)
                    yt2 = pool.tile([128, ow], F32, name=f"yt2{oc}_{hf}_{j}",
                                    tag="yt2", bufs=3)
                    nc.vector.tensor_tensor(yt2[:], yt[:], bst[:], OP.add)
                    n0 = base + j * 128
                    nc.sync.dma_start(y[n0:n0 + 128, o0:o0 + ow], yt2[:])



import numpy as np

NCORES = 8
B, S, H, O, E = 4, 2048, 4096, 4096, 4
N = B * S
Nc = N // NCORES
CFG = dict(H=H, O=O, Nc=Nc, E=E, half=512, ow=512)

TRACE = False
LAST_EXEC_NS = None
LAST_TRACE_PATH = None
_NC_CACHE = None


def _get_nc():
    global _NC_CACHE
    if _NC_CACHE is None:
        import concourse.bacc as bacc
        import concourse.tile as tile
        nc = bacc.Bacc("TRN2", target_bir_lowering=False, debug=False,
                       num_devices=NCORES)
        ins_aps = {
            "xT": nc.dram_tensor("xT", [H, Nc], F32R, kind="ExternalInput").ap(),
            "wT": nc.dram_tensor("wT", [H, O], F32R, kind="ExternalInput").ap(),
            "gwT": nc.dram_tensor("gwT", [H, E], F32R, kind="ExternalInput").ap(),
            "ics": nc.dram_tensor("ics", [E, H], F32R, kind="ExternalInput").ap(),
            "ocsb": nc.dram_tensor("ocsb", [E * 128, O], F32, kind="ExternalInput").ap(),
            "bias2": nc.dram_tensor("bias2", [128, O], F32, kind="ExternalInput").ap(),
            "ones_e": nc.dram_tensor("ones_e", [E, E], F32R, kind="ExternalInput").ap(),
            "eye_e": nc.dram_tensor("eye_e", [E, E], F32R, kind="ExternalInput").ap(),
        }
        outs_aps = {"y": nc.dram_tensor("y", [Nc, O], F32, kind="ExternalOutput").ap()}
        with tile.TileContext(nc) as tc:
            with ExitStack() as ctx:
                build_moe8(ctx, tc, outs_aps, ins_aps, CFG)
        nc.compile()
        _NC_CACHE = nc
    return _NC_CACHE


def kernel(x, weight, bias, gate_w, in_channel_scale, out_channel_scale):
    """Full inputs in, full output out; distributes over 8 NeuronCores."""
    global LAST_EXEC_NS, LAST_TRACE_PATH
    from concourse.bass_utils import run_bass_kernel_spmd

    x = np.asarray(x, dtype=np.float32)
    weight = np.asarray(weight, dtype=np.float32)
    bias = np.asarray(bias, dtype=np.float32)
    gate_w = np.asarray(gate_w, dtype=np.float32)
    ics = np.asarray(in_channel_scale, dtype=np.float32)
    ocs = np.asarray(out_channel_scale, dtype=np.float32)

    nc = _get_nc()
    xf = np.ascontiguousarray(x.reshape(N, H))
    wTc = np.ascontiguousarray(weight.T)
    gwTc = np.ascontiguousarray(gate_w.T)
    bias2 = np.ascontiguousarray(np.broadcast_to(bias[None, :], (128, O)))
    ocsb = np.ascontiguousarray(
        np.broadcast_to(ocs[:, None, :], (E, 128, O)).reshape(E * 128, O))
    in_maps = []
    for c in range(NCORES):
        in_maps.append({
            "xT": np.ascontiguousarray(xf[c * Nc:(c + 1) * Nc, :].T),
            "wT": wTc, "gwT": gwTc, "ics": ics, "ocsb": ocsb, "bias2": bias2,
            "ones_e": np.ones((E, E), dtype=np.float32),
            "eye_e": np.eye(E, dtype=np.float32),
        })
    res = run_bass_kernel_spmd(nc, in_maps, core_ids=list(range(NCORES)),
                               trace=TRACE)
    if TRACE:
        LAST_EXEC_NS = res.exec_time_ns
        if res.instructions_and_trace:
            LAST_TRACE_PATH = res.instructions_and_trace[1]
    yfull = np.concatenate([res.results[c]["y"] for c in range(NCORES)], axis=0)
    return yfull.reshape(B, S, O)

